# revision 1
# baseline (speedup 1.0000x reference)
"""CollisionRegularizer loss on 8 Trainium2 cores.

Strategy: every pairwise quantity (dist^2, the 6 scaled rotated-radius
projections, the velocity-approach dot) is a low-rank bilinear form in
per-point features, so they are computed as small-K matmuls on the PE
(host-prepped augmented feature rows), followed by a short elementwise
chain on DVE/ACT. Sharding: core c handles batch c//2, row-half c%2.
Each core emits per-partition partial sums; the host reduces.
"""

import numpy as np

import concourse.bacc as bacc
import concourse.mybir as mybir
from concourse import tile
from concourse.bass_utils import run_bass_kernel_spmd

B, N = 4, 2048
NC = 8
ROWS = 1024          # n-rows per core
NT = ROWS // 128     # 8 partition tiles
MC = 2               # m-chunks per row-tile
CHUNK = N // MC      # 1024 free-dim chain width
F32 = mybir.dt.float32

MM_TYPES = ["d2", "va", "su0", "su1", "su2", "sv0", "sv1", "sv2"]


def _quat_to_rotmat(q):
    qw, qx, qy, qz = q[..., 0], q[..., 1], q[..., 2], q[..., 3]
    R = np.stack(
        [
            1 - 2 * qy**2 - 2 * qz**2, 2 * qx * qy - 2 * qz * qw, 2 * qx * qz + 2 * qy * qw,
            2 * qx * qy + 2 * qz * qw, 1 - 2 * qx**2 - 2 * qz**2, 2 * qy * qz - 2 * qx * qw,
            2 * qx * qz - 2 * qy * qw, 2 * qy * qz + 2 * qx * qw, 1 - 2 * qx**2 - 2 * qy**2,
        ],
        axis=-1,
    )
    return R.reshape(*q.shape[:-1], 3, 3)


def _prep(xyz, scales, rotations, velocities):
    x = xyz.astype(np.float64)
    s = scales.astype(np.float64)
    v = velocities.astype(np.float64)
    R = _quat_to_rotmat(rotations.astype(np.float64))      # (B,N,3,3)
    a = np.einsum("bni,bnij->bnj", x, R)                   # x_n . R[n][:,j]
    c = (v * x).sum(-1)                                    # v_n . x_n
    nrm = (x * x).sum(-1)

    rhs = np.empty((B, 33, N), np.float32)
    rhs[:, 0:3] = x.transpose(0, 2, 1)
    rhs[:, 3] = 1.0
    rhs[:, 4] = nrm
    rhs[:, 5:8] = v.transpose(0, 2, 1)
    rhs[:, 8] = c
    for j in range(3):
        b0 = 9 + 4 * j
        rhs[:, b0:b0 + 3] = (x * s[:, :, j:j + 1]).transpose(0, 2, 1)
        rhs[:, b0 + 3] = s[:, :, j]
    for j in range(3):
        b0 = 21 + 4 * j
        rhs[:, b0:b0 + 3] = R[:, :, :, j].transpose(0, 2, 1)
        rhs[:, b0 + 3] = a[:, :, j]

    lhs = np.zeros((B, 8, 33, N), np.float32)
    lhs[:, 0, 0:3] = (-2.0 * x).transpose(0, 2, 1)
    lhs[:, 0, 3] = nrm + 1e-8
    lhs[:, 0, 4] = 1.0
    lhs[:, 1, 0:3] = v.transpose(0, 2, 1)
    lhs[:, 1, 3] = -c
    lhs[:, 1, 5:8] = x.transpose(0, 2, 1)
    lhs[:, 1, 8] = -1.0
    for j in range(3):
        b0 = 9 + 4 * j
        lhs[:, 2 + j, b0:b0 + 3] = R[:, :, :, j].transpose(0, 2, 1)
        lhs[:, 2 + j, b0 + 3] = -a[:, :, j]
    for j in range(3):
        b0 = 21 + 4 * j
        lhs[:, 5 + j, b0:b0 + 3] = (x * s[:, :, j:j + 1]).transpose(0, 2, 1)
        lhs[:, 5 + j, b0 + 3] = -s[:, :, j]
    return rhs, lhs


_NC_CACHE = {}

# perf config
F32R = True        # reduced-precision single-pass fp32 matmuls on PE
CHAIN_FP16 = True  # fp16 elementwise chain (2x/4x DVE throughput)
CLAMP = 1e-4       # dist^2 floor; keeps duplicates/diagonal harmless and
                   # bounds inv<=100 so every fp16 intermediate stays in range
F16 = mybir.dt.float16

# engine assignment for flexible elementwise ops: "dve" or "pool"
ASSIGN = {
    "add_r1s": "dve", "add_r2s": "dve", "rsum": "dve",
    "t": "dve", "ovp": "dve", "ov": "act", "den": "dve",
    "sqov": "dve", "g": "dve",
}


def _build(reps=1):
    key = (reps, F32R, CHAIN_FP16, tuple(sorted(ASSIGN.items())))
    if key in _NC_CACHE:
        return _NC_CACHE[key]
    CT = F16 if CHAIN_FP16 else F32
    MMT = mybir.dt.float32r if F32R else F32
    AF = mybir.ActivationFunctionType
    nc = bacc.Bacc(None, target_bir_lowering=False, debug=False)

    def _eng(k):
        return nc.gpsimd if ASSIGN[k] == "pool" else nc.vector

    rhs_d = nc.dram_tensor("rhs", [33, N], MMT, kind="ExternalInput")
    lhs_d = nc.dram_tensor("lhs", [8, 33, ROWS], MMT, kind="ExternalInput")
    rhs32_d = nc.dram_tensor("rhs32", [5, N], F32, kind="ExternalInput")
    lhs32_d = nc.dram_tensor("lhs32", [5, ROWS], F32, kind="ExternalInput")
    out_d = nc.dram_tensor("out", [128, 2 * NT * MC], F32, kind="ExternalOutput")

    with tile.TileContext(nc) as tc:
        with (
            tc.tile_pool(name="io", bufs=1) as io,
            tc.tile_pool(name="wk", bufs=3) as wk,
            tc.tile_pool(name="ch", bufs=3) as ch,
            tc.tile_pool(name="ps", bufs=4, space="PSUM") as ps,
        ):
            rhs_s = io.tile([33, N], MMT)
            nc.sync.dma_start(rhs_s[:], rhs_d[:])
            lhs_t = {}
            for ti, name in enumerate(MM_TYPES):
                lhs_t[name] = io.tile([33, ROWS], MMT, name="lhs_" + name)
                nc.sync.dma_start(lhs_t[name][:], lhs_d[ti])
            rhs32_s = io.tile([5, N], F32)
            nc.sync.dma_start(rhs32_s[:], rhs32_d[:])
            lhs32_s = io.tile([5, ROWS], F32)
            nc.sync.dma_start(lhs32_s[:], lhs32_d[:])
            ocols = io.tile([128, 2 * NT * MC], F32)

            from contextlib import nullcontext
            loop_cm = tc.For_i(0, reps, 1) if reps > 1 else nullcontext()
            with loop_cm:
              for nt in range(NT):
                nsl = slice(nt * 128, (nt + 1) * 128)
                for mc in range(MC):
                    it = nt * MC + mc
                    pt = {}
                    for name in MM_TYPES:
                        p = ps.tile([128, CHUNK], F32, name="p_" + name, tag="mm")
                        for h in range(CHUNK // 512):
                            m0 = mc * CHUNK + h * 512
                            if name == "d2":
                                nc.tensor.matmul(
                                    p[:, h * 512:(h + 1) * 512],
                                    lhs32_s[:, nsl],
                                    rhs32_s[:, m0:m0 + 512],
                                    start=True, stop=True,
                                )
                            else:
                                nc.tensor.matmul(
                                    p[:, h * 512:(h + 1) * 512],
                                    lhs_t[name][:, nsl],
                                    rhs_s[:, m0:m0 + 512],
                                    start=True, stop=True,
                                )
                        pt[name] = p

                    # PSUM drains
                    d2c = wk.tile([128, CHUNK], CT)
                    nc.vector.tensor_scalar_max(d2c[:], pt["d2"][:], CLAMP)
                    rva = wk.tile([128, CHUNK], CT)
                    nc.scalar.activation(rva[:], pt["va"][:], AF.Relu, scale=0.1)
                    # r1s via ACT squares (DVE cannot square PSUM) + Pool adds
                    squ = []
                    for j in range(3):
                        sq = wk.tile([128, CHUNK], CT, name=f"squ{j}")
                        nc.scalar.activation(sq[:], pt[f"su{j}"][:], AF.Square)
                        squ.append(sq)
                    r1s = wk.tile([128, CHUNK], CT)
                    _eng("add_r1s").tensor_add(r1s[:], squ[0][:], squ[1][:])
                    _eng("add_r1s").tensor_add(r1s[:], r1s[:], squ[2][:])
                    # r2s via ACT squares + Pool adds
                    sqv = []
                    for j in range(3):
                        sq = wk.tile([128, CHUNK], CT, name=f"sqv{j}")
                        nc.scalar.activation(sq[:], pt[f"sv{j}"][:], AF.Square)
                        sqv.append(sq)
                    r2s = wk.tile([128, CHUNK], CT)
                    _eng("add_r2s").tensor_add(r2s[:], sqv[0][:], sqv[1][:])
                    _eng("add_r2s").tensor_add(r2s[:], r2s[:], sqv[2][:])

                    dist = wk.tile([128, CHUNK], CT)
                    nc.scalar.activation(dist[:], d2c[:], AF.Sqrt)
                    inv = wk.tile([128, CHUNK], CT)
                    with nc.allow_low_precision("fp16 chain: inv<=100, rel err 5e-4"):
                        nc.vector.reciprocal(inv[:], dist[:])
                    r1 = ch.tile([128, CHUNK], CT)
                    nc.scalar.activation(r1[:], r1s[:], AF.Sqrt)
                    r2 = ch.tile([128, CHUNK], CT)
                    nc.scalar.activation(r2[:], r2s[:], AF.Sqrt)

                    rsum = ch.tile([128, CHUNK], CT)
                    _eng("rsum").tensor_add(rsum[:], r1[:], r2[:])
                    t = ch.tile([128, CHUNK], CT)
                    _eng("t").tensor_mul(t[:], rsum[:], inv[:])
                    ovp = ch.tile([128, CHUNK], CT)
                    _eng("ovp").tensor_sub(ovp[:], t[:], dist[:])
                    ov = wk.tile([128, CHUNK], CT)
                    if ASSIGN["ov"] == "act":
                        nc.scalar.activation(ov[:], ovp[:], AF.Relu)
                    else:
                        _eng("ov").tensor_scalar_max(ov[:], ovp[:], 0.0)

                    den = ch.tile([128, CHUNK], CT)
                    if ASSIGN["den"] == "act":
                        nc.scalar.activation(den[:], ov[:], AF.Identity,
                                             bias=1.0, scale=0.1)
                    else:
                        _eng("den").tensor_scalar(den[:], ov[:], 0.1, 1.0,
                                                  mybir.AluOpType.mult,
                                                  mybir.AluOpType.add)
                    rden = ch.tile([128, CHUNK], CT)
                    with nc.allow_low_precision("fp16 chain"):
                        nc.vector.reciprocal(rden[:], den[:])
                    sqov = ch.tile([128, CHUNK], CT)
                    if ASSIGN["sqov"] == "act":
                        nc.scalar.activation(sqov[:], ov[:], AF.Square)
                    else:
                        _eng("sqov").tensor_mul(sqov[:], ov[:], ov[:])
                    spec = ch.tile([128, CHUNK], CT)
                    nc.vector.scalar_tensor_tensor(
                        out=spec[:], in0=sqov[:], scalar=1.0, in1=rden[:],
                        op0=mybir.AluOpType.mult, op1=mybir.AluOpType.mult,
                        accum_out=ocols[:, 2 * it:2 * it + 1])

                    g = ch.tile([128, CHUNK], CT)
                    _eng("g").tensor_mul(g[:], ov[:], inv[:])
                    vt = ch.tile([128, CHUNK], CT)
                    nc.vector.scalar_tensor_tensor(
                        out=vt[:], in0=g[:], scalar=1.0, in1=rva[:],
                        op0=mybir.AluOpType.mult, op1=mybir.AluOpType.mult,
                        accum_out=ocols[:, 2 * it + 1:2 * it + 2])

            nc.sync.dma_start(out_d[:], ocols[:])

    nc.compile()
    _NC_CACHE[key] = nc
    return nc


def make_in_maps(xyz, scales, rotations, velocities):
    rhs, lhs = _prep(xyz, scales, rotations, velocities)
    in_maps = []
    for c in range(NC):
        b, half = c // 2, c % 2
        in_maps.append({
            "rhs": np.ascontiguousarray(rhs[b]),
            "lhs": np.ascontiguousarray(lhs[b][:, :, half * ROWS:(half + 1) * ROWS]),
            "rhs32": np.ascontiguousarray(rhs[b][0:5]),
            "lhs32": np.ascontiguousarray(lhs[b][0, 0:5, half * ROWS:(half + 1) * ROWS]),
        })
    return in_maps


def finish(results):
    total = 0.0
    for c in range(NC):
        total += results[c]["out"].astype(np.float64).sum()
    return np.float32(total / (B * N * N))


_RUNNER = {}


def _get_runner(reps=1):
    """Cached shard_map-jitted executor (mirrors bass2jax.run_bass_via_pjrt
    multi-core path) so repeated calls skip re-compilation."""
    if reps in _RUNNER:
        return _RUNNER[reps]
    import jax
    from jax.sharding import Mesh, PartitionSpec
    from jax.experimental.shard_map import shard_map
    from concourse import bass2jax

    nc = _build(reps)
    bass2jax.install_neuronx_cc_hook()

    part_name = nc.partition_id_tensor.name if nc.partition_id_tensor else None
    in_names, out_names, out_avals, zero_outs = [], [], [], []
    for alloc in nc.m.functions[0].allocations:
        if not isinstance(alloc, mybir.MemoryLocationSet):
            continue
        name = alloc.memorylocations[0].name
        if alloc.kind == "ExternalInput":
            if name != part_name:
                in_names.append(name)
        elif alloc.kind == "ExternalOutput":
            out_names.append(name)
            shape = tuple(alloc.tensor_shape)
            dtype = mybir.dt.np(alloc.dtype)
            out_avals.append(jax.core.ShapedArray(shape, dtype))
            zero_outs.append(np.zeros(shape, dtype))
    n_params = len(in_names)
    all_names = in_names + out_names
    if part_name is not None:
        all_names = all_names + [part_name]

    def _body(*args):
        operands = list(args)
        if part_name is not None:
            operands.append(bass2jax.partition_id_tensor())
        outs = bass2jax._bass_exec_p.bind(
            *operands,
            out_avals=tuple(out_avals),
            in_names=tuple(all_names),
            out_names=tuple(out_names),
            lowering_input_output_aliases=(),
            sim_require_finite=True,
            sim_require_nnan=True,
            nc=nc,
        )
        return tuple(outs)

    devices = jax.devices()[:NC]
    mesh = Mesh(np.asarray(devices), ("core",))
    n_outs = len(out_names)
    fn = jax.jit(
        shard_map(
            _body, mesh=mesh,
            in_specs=(PartitionSpec("core"),) * (n_params + n_outs),
            out_specs=(PartitionSpec("core"),) * n_outs,
            check_rep=False,
        ),
        donate_argnums=tuple(range(n_params, n_params + n_outs)),
        keep_unused=True,
    )

    def run(in_maps):
        concat_in = [
            np.concatenate([in_maps[c][nm] for c in range(NC)], axis=0)
            for nm in in_names
        ]
        concat_zeros = [
            np.zeros((NC * z.shape[0], *z.shape[1:]), z.dtype) for z in zero_outs
        ]
        out_arrs = fn(*concat_in, *concat_zeros)
        return [
            {nm: np.asarray(out_arrs[i]).reshape(NC, *out_avals[i].shape)[c]
             for i, nm in enumerate(out_names)}
            for c in range(NC)
        ]

    _RUNNER[reps] = run
    return run


def kernel(xyz, scales, rotations, velocities):
    run = _get_runner()
    in_maps = make_in_maps(xyz, scales, rotations, velocities)
    return finish(run(in_maps))


if __name__ == "__main__":
    rng = np.random.default_rng(0)
    ins = {
        "xyz": rng.standard_normal((B, N, 3)).astype(np.float32),
        "scales": rng.random((B, N, 3)).astype(np.float32),
        "rotations": rng.standard_normal((B, N, 4)).astype(np.float32),
        "velocities": rng.standard_normal((B, N, 3)).astype(np.float32),
    }
    print(kernel(**ins))



# revision 2
# speedup vs baseline: 4.1683x; 4.1683x over previous
"""CollisionRegularizer loss on 8 Trainium2 cores — v2.

Key ideas vs the v1 baseline (408us):
  * r_directional^2 for both pair orderings is computed DIRECTLY as a
    K=30 bilinear form on the PE (quadratic-form features prepped on
    host), eliminating the 6 projection matmuls + 6 ACT squares + 4 DVE
    adds per tile of v1.
  * Pair matrix is symmetric (spec and the masked-velocity integrand are
    both symmetric in (n,m)), so only the upper block-triangle is
    computed: row-tile k covers columns 128k..2048.  Off-diagonal blocks
    are counted twice on the host; true diagonal is zeroed with a
    128x128 (1-I) mask multiply on the fp16 chain.
  * Slow `nc.vector.reciprocal` (iterative divide) replaced by the
    single-instruction RECIPROCAL_APPROX_FAST custom DVE op (fp16
    in/out via direct _custom_dve emission).
  * spec/vt accumulations ride scalar_tensor_tensor accum_out (4x-mode
    capable InstTensorScalarPtr).
  * All 8 cores run ONE program with fixed slot widths W(ti)=2048-256*ti;
    odd row-halves are shifted by 128 columns by the host and see 128
    sentinel pad columns (distant points, zero scales/velocities) that
    contribute exactly 0.

Sharding: core c -> batch c//2, row-half c%2; slot ti -> row-tile
k = 2*ti + half (rows 128k..128k+128).
"""

import numpy as np

import concourse.bacc as bacc
import concourse.mybir as mybir
from concourse import tile

B, N = 4, 2048
NC = 8
NSLOT = 8
ROWS = 1024           # 8 row-tiles of 128 per core
PADC = 256            # sentinel pad columns appended to rhs (global)
RHS_COLS = 2176       # per-core rhs view width
S16 = 1.0 / 16.0      # r1s/r2s prescale so fp16 stays in range
D2_BIAS = 1e-4        # distance^2 floor (replaces diagonal clamp)

F32 = mybir.dt.float32
F32R = mybir.dt.float32r
F16 = mybir.dt.float16

# engine assignment for tunable ops: 'dve' | 'act' | 'pool'
ASSIGN = {
    "r1c": "act",    # clamp of r1s psum (relu)
    "r2c": "dve",    # clamp of r2s psum (tensor_scalar max)
    "rsum": "pool",
    "ovp": "pool",
    "t": "dve",
    "sqov": "dve",
    "g": "dve",
    "den": "dve",
}


def _slot_w(ti):
    return 2048 - 256 * ti


def _chunks(w):
    out = []
    off = 0
    while off < w:
        cw = min(512, w - off)
        out.append((off, cw))
        off += cw
    return out


def _quat_to_rotmat(q):
    qw, qx, qy, qz = q[..., 0], q[..., 1], q[..., 2], q[..., 3]
    R = np.stack(
        [
            1 - 2 * qy**2 - 2 * qz**2, 2 * qx * qy - 2 * qz * qw, 2 * qx * qz + 2 * qy * qw,
            2 * qx * qy + 2 * qz * qw, 1 - 2 * qx**2 - 2 * qz**2, 2 * qy * qz - 2 * qx * qw,
            2 * qx * qz - 2 * qy * qw, 2 * qy * qz + 2 * qx * qw, 1 - 2 * qx**2 - 2 * qy**2,
        ],
        axis=-1,
    )
    return R.reshape(*q.shape[:-1], 3, 3)


def _xx6(x):
    return np.stack(
        [x[:, 0] * x[:, 0], x[:, 0] * x[:, 1], x[:, 0] * x[:, 2],
         x[:, 1] * x[:, 1], x[:, 1] * x[:, 2], x[:, 2] * x[:, 2]], 0)


def _rr10(Rcol, aj):
    # [RR6 with doubled cross terms, -2*aj*Rcol, aj^2] -> (10, N)
    r0, r1, r2 = Rcol[:, 0], Rcol[:, 1], Rcol[:, 2]
    return np.stack(
        [r0 * r0, 2 * r0 * r1, 2 * r0 * r2, r1 * r1, 2 * r1 * r2, r2 * r2,
         -2 * aj * r0, -2 * aj * r1, -2 * aj * r2, aj * aj], 0)


def _sfeat10(x, s2j):
    # [s2j*xx6, s2j*x, s2j] -> (10, N)
    return np.concatenate([s2j * _xx6(x), s2j * x.T, s2j[None, :] * np.ones((1, x.shape[0]))], 0)


def _prep(xyz, scales, rotations, velocities):
    """Per-batch padded feature stacks (float64 host math, f32 out)."""
    NB = N + PADC
    rhs_d2 = np.zeros((B, 5, NB), np.float64)
    rhs_va = np.zeros((B, 8, NB), np.float64)
    rhs_r1 = np.zeros((B, 30, NB), np.float64)
    rhs_r2 = np.zeros((B, 30, NB), np.float64)
    lhs_d2 = np.zeros((B, 5, N), np.float64)
    lhs_va = np.zeros((B, 8, N), np.float64)
    lhs_r1 = np.zeros((B, 30, N), np.float64)
    lhs_r2 = np.zeros((B, 30, N), np.float64)

    # sentinel pad point: far away, zero scales/velocities, identity R
    xp = np.zeros((PADC, 3)); xp[:, 0] = 80.0
    sp = np.zeros((PADC, 3)); vp = np.zeros((PADC, 3))
    Rp = np.broadcast_to(np.eye(3), (PADC, 3, 3))

    for b in range(B):
        x = xyz[b].astype(np.float64)
        s = scales[b].astype(np.float64)
        v = velocities[b].astype(np.float64)
        R = _quat_to_rotmat(rotations[b].astype(np.float64))

        xa = np.concatenate([x, xp], 0)
        sa = np.concatenate([s, sp], 0)
        va_ = np.concatenate([v, vp], 0)
        Ra = np.concatenate([R, Rp], 0)

        aa = np.einsum("ni,nij->nj", xa, Ra)
        ca = (va_ * xa).sum(-1)
        nrma = (xa * xa).sum(-1)
        s2a = sa * sa

        rhs_d2[b, 0:3] = xa.T
        rhs_d2[b, 3] = 1.0
        rhs_d2[b, 4] = nrma
        rhs_va[b, 0:3] = xa.T
        rhs_va[b, 3] = 1.0
        rhs_va[b, 4:7] = va_.T
        rhs_va[b, 7] = ca
        for j in range(3):
            rhs_r1[b, 10 * j:10 * j + 10] = _sfeat10(xa, s2a[:, j])
            rhs_r2[b, 10 * j:10 * j + 10] = _rr10(Ra[:, :, j], aa[:, j])

        a = aa[:N]; c = ca[:N]; nrm = nrma[:N]; s2 = s2a[:N]
        lhs_d2[b, 0:3] = -2.0 * x.T
        lhs_d2[b, 3] = nrm + D2_BIAS
        lhs_d2[b, 4] = 1.0
        lhs_va[b, 0:3] = v.T
        lhs_va[b, 3] = -c
        lhs_va[b, 4:7] = x.T
        lhs_va[b, 7] = -1.0
        for j in range(3):
            lhs_r1[b, 10 * j:10 * j + 10] = _rr10(R[:, :, j], a[:, j]) * S16
            lhs_r2[b, 10 * j:10 * j + 10] = _sfeat10(x, s2[:, j]) * S16

    f = np.float32
    return (f(rhs_d2), f(rhs_va), f(rhs_r1), f(rhs_r2),
            f(lhs_d2), f(lhs_va), f(lhs_r1), f(lhs_r2))


_NC_CACHE = {}


def _build(reps=1):
    key = (reps, tuple(sorted(ASSIGN.items())))
    if key in _NC_CACHE:
        return _NC_CACHE[key]
    AF = mybir.ActivationFunctionType
    ALU = mybir.AluOpType
    from concourse.dve_ops import (
        RECIP_APPROX_FAST_CONSTS as RC,
        RECIPROCAL_APPROX_FAST,
    )
    nc = bacc.Bacc(None, target_bir_lowering=False, debug=False)

    rhs_d2_d = nc.dram_tensor("rhs_d2", [5, RHS_COLS], F32, kind="ExternalInput")
    rhs_va_d = nc.dram_tensor("rhs_va", [8, RHS_COLS], F32R, kind="ExternalInput")
    rhs_r1_d = nc.dram_tensor("rhs_r1", [30, RHS_COLS], F32R, kind="ExternalInput")
    rhs_r2_d = nc.dram_tensor("rhs_r2", [30, RHS_COLS], F32R, kind="ExternalInput")
    lhs_d2_d = nc.dram_tensor("lhs_d2", [5, ROWS], F32, kind="ExternalInput")
    lhs_va_d = nc.dram_tensor("lhs_va", [8, ROWS], F32R, kind="ExternalInput")
    lhs_r1_d = nc.dram_tensor("lhs_r1", [30, ROWS], F32R, kind="ExternalInput")
    lhs_r2_d = nc.dram_tensor("lhs_r2", [30, ROWS], F32R, kind="ExternalInput")
    dmask_d = nc.dram_tensor("dmask", [128, 128], F16, kind="ExternalInput")
    out_d = nc.dram_tensor("out", [128, 4 * NSLOT], F32, kind="ExternalOutput")

    def _recip_fast(eng, out, in_):
        return eng._custom_dve(
            RECIPROCAL_APPROX_FAST, out=out, in0=in_,
            s0=RC["s0"], s1=RC["s1"], imm2=RC["imm2"])

    with tile.TileContext(nc) as tc:
        with (
            tc.tile_pool(name="io", bufs=1) as io,
            tc.tile_pool(name="wk", bufs=2) as wk,
            tc.tile_pool(name="ps", bufs=8, space="PSUM") as ps,
        ):
            rhs_d2_s = io.tile([5, RHS_COLS], F32)
            nc.sync.dma_start(rhs_d2_s[:], rhs_d2_d[:])
            rhs_va_s = io.tile([8, RHS_COLS], F32R)
            nc.sync.dma_start(rhs_va_s[:], rhs_va_d[:])
            rhs_r1_s = io.tile([30, RHS_COLS], F32R)
            nc.sync.dma_start(rhs_r1_s[:], rhs_r1_d[:])
            rhs_r2_s = io.tile([30, RHS_COLS], F32R)
            nc.sync.dma_start(rhs_r2_s[:], rhs_r2_d[:])
            lhs_d2_s = io.tile([5, ROWS], F32)
            nc.sync.dma_start(lhs_d2_s[:], lhs_d2_d[:])
            lhs_va_s = io.tile([8, ROWS], F32R)
            nc.sync.dma_start(lhs_va_s[:], lhs_va_d[:])
            lhs_r1_s = io.tile([30, ROWS], F32R)
            nc.sync.dma_start(lhs_r1_s[:], lhs_r1_d[:])
            lhs_r2_s = io.tile([30, ROWS], F32R)
            nc.sync.dma_start(lhs_r2_s[:], lhs_r2_d[:])
            dmask_s = io.tile([128, 128], F16)
            nc.sync.dma_start(dmask_s[:], dmask_d[:])
            ocols = io.tile([128, 4 * NSLOT], F32)

            from contextlib import nullcontext
            loop_cm = tc.For_i(0, reps, 1) if reps > 1 else nullcontext()
            with loop_cm:
              for ti in range(NSLOT):
                W = _slot_w(ti)
                rsl = slice(ti * 128, ti * 128 + 128)
                cbase = 256 * ti

                dist = wk.tile([128, W], F16, name="dist", tag="dist")
                r1c = wk.tile([128, W], F16, name="r1c", tag="r1c")
                r2c = wk.tile([128, W], F16, name="r2c", tag="r2c")
                rva = wk.tile([128, W], F16, name="rva", tag="rva")

                for (off, cw) in _chunks(W):
                    gsl = slice(cbase + off, cbase + off + cw)
                    osl = slice(off, off + cw)
                    pd2 = ps.tile([128, cw], F32, name="pd2", tag="mm")
                    nc.tensor.matmul(pd2[:], lhs_d2_s[:, rsl], rhs_d2_s[:, gsl],
                                     start=True, stop=True)
                    pr1 = ps.tile([128, cw], F32, name="pr1", tag="mm")
                    nc.tensor.matmul(pr1[:], lhs_r1_s[:, rsl], rhs_r1_s[:, gsl],
                                     start=True, stop=True)
                    pr2 = ps.tile([128, cw], F32, name="pr2", tag="mm")
                    nc.tensor.matmul(pr2[:], lhs_r2_s[:, rsl], rhs_r2_s[:, gsl],
                                     start=True, stop=True)
                    pva = ps.tile([128, cw], F32, name="pva", tag="mm")
                    nc.tensor.matmul(pva[:], lhs_va_s[:, rsl], rhs_va_s[:, gsl],
                                     start=True, stop=True)

                    nc.scalar.activation(dist[:, osl], pd2[:], AF.Sqrt)
                    if ASSIGN["r1c"] == "act":
                        nc.scalar.activation(r1c[:, osl], pr1[:], AF.Relu)
                    else:
                        nc.vector.tensor_scalar_max(r1c[:, osl], pr1[:], 0.0)
                    if ASSIGN["r2c"] == "act":
                        nc.scalar.activation(r2c[:, osl], pr2[:], AF.Relu)
                    else:
                        nc.vector.tensor_scalar_max(r2c[:, osl], pr2[:], 0.0)
                    nc.scalar.activation(rva[:, osl], pva[:], AF.Relu, scale=0.1)

                # wide fp16 chain
                r1 = wk.tile([128, W], F16, name="r1", tag="r1")
                nc.scalar.activation(r1[:], r1c[:], AF.Sqrt, scale=16.0)
                r2 = wk.tile([128, W], F16, name="r2", tag="r2")
                nc.scalar.activation(r2[:], r2c[:], AF.Sqrt, scale=16.0)
                inv = wk.tile([128, W], F16, name="inv", tag="inv")
                with nc.allow_low_precision("fp16 chain, ~51ULP recip"):
                    _recip_fast(nc.vector, inv[:], dist[:])

                rsum = wk.tile([128, W], F16, name="rsum", tag="rsum")
                _eng = lambda k: {"dve": nc.vector, "pool": nc.gpsimd}[ASSIGN[k]]
                _eng("rsum").tensor_add(rsum[:], r1[:], r2[:])
                t = wk.tile([128, W], F16, name="t", tag="t")
                _eng("t").tensor_mul(t[:], rsum[:], inv[:])
                ovp = wk.tile([128, W], F16, name="ovp", tag="ovp")
                _eng("ovp").tensor_sub(ovp[:], t[:], dist[:])
                ov = wk.tile([128, W], F16, name="ov", tag="ov")
                nc.vector.tensor_scalar_max(ov[:], ovp[:], 0.0)
                # zero the true diagonal (block-local cols 0..128)
                nc.vector.tensor_mul(ov[:, 0:128], ov[:, 0:128], dmask_s[:])

                den = wk.tile([128, W], F16, name="den", tag="den")
                if ASSIGN["den"] == "act":
                    nc.scalar.activation(den[:], ov[:], AF.Identity,
                                         bias=1.0, scale=0.1)
                else:
                    nc.vector.tensor_scalar(den[:], ov[:], 0.1, 1.0,
                                            ALU.mult, ALU.add)
                rden = wk.tile([128, W], F16, name="rden", tag="rden")
                with nc.allow_low_precision("fp16 chain"):
                    _recip_fast(nc.vector, rden[:], den[:])
                sqov = wk.tile([128, W], F16, name="sqov", tag="sqov")
                _eng("sqov").tensor_mul(sqov[:], ov[:], ov[:])
                g = wk.tile([128, W], F16, name="g", tag="g")
                _eng("g").tensor_mul(g[:], ov[:], inv[:])

                scr = wk.tile([128, W], F16, name="scr", tag="scr")
                # spec = ov^2 * rden ; vt = (ov/dist) * relu(0.1*va)
                nc.vector.scalar_tensor_tensor(
                    out=scr[:, 0:128], in0=sqov[:, 0:128], scalar=1.0,
                    in1=rden[:, 0:128], op0=ALU.mult, op1=ALU.mult,
                    accum_out=ocols[:, 4 * ti:4 * ti + 1])
                nc.vector.scalar_tensor_tensor(
                    out=scr[:, 128:W], in0=sqov[:, 128:W], scalar=1.0,
                    in1=rden[:, 128:W], op0=ALU.mult, op1=ALU.mult,
                    accum_out=ocols[:, 4 * ti + 1:4 * ti + 2])
                nc.vector.scalar_tensor_tensor(
                    out=scr[:, 0:128], in0=g[:, 0:128], scalar=1.0,
                    in1=rva[:, 0:128], op0=ALU.mult, op1=ALU.mult,
                    accum_out=ocols[:, 4 * ti + 2:4 * ti + 3])
                nc.vector.scalar_tensor_tensor(
                    out=scr[:, 128:W], in0=g[:, 128:W], scalar=1.0,
                    in1=rva[:, 128:W], op0=ALU.mult, op1=ALU.mult,
                    accum_out=ocols[:, 4 * ti + 3:4 * ti + 4])

            nc.sync.dma_start(out_d[:], ocols[:])

    nc.compile()
    _NC_CACHE[key] = nc
    return nc


def make_in_maps(xyz, scales, rotations, velocities):
    rhs_d2, rhs_va, rhs_r1, rhs_r2, lhs_d2, lhs_va, lhs_r1, lhs_r2 = _prep(
        xyz, scales, rotations, velocities)
    dmask = (1.0 - np.eye(128)).astype(np.float16)
    in_maps = []
    for c in range(NC):
        b, half = c // 2, c % 2
        csl = slice(128 * half, 128 * half + RHS_COLS)
        # lhs rows: slot ti -> row-tile k = 2*ti + half
        def lrows(a):
            return np.ascontiguousarray(np.concatenate(
                [a[:, 128 * (2 * ti + half):128 * (2 * ti + half) + 128]
                 for ti in range(NSLOT)], axis=1))
        in_maps.append({
            "rhs_d2": np.ascontiguousarray(rhs_d2[b][:, csl]),
            "rhs_va": np.ascontiguousarray(rhs_va[b][:, csl]),
            "rhs_r1": np.ascontiguousarray(rhs_r1[b][:, csl]),
            "rhs_r2": np.ascontiguousarray(rhs_r2[b][:, csl]),
            "lhs_d2": lrows(lhs_d2[b]),
            "lhs_va": lrows(lhs_va[b]),
            "lhs_r1": lrows(lhs_r1[b]),
            "lhs_r2": lrows(lhs_r2[b]),
            "dmask": dmask,
        })
    return in_maps


def finish(results):
    total = 0.0
    for c in range(NC):
        o = results[c]["out"].astype(np.float64)  # [128, 4*NSLOT]
        for ti in range(NSLOT):
            total += o[:, 4 * ti].sum() + 2.0 * o[:, 4 * ti + 1].sum()
            total += o[:, 4 * ti + 2].sum() + 2.0 * o[:, 4 * ti + 3].sum()
    return np.float32(total / (B * N * N))


_RUNNER = {}


def _get_runner(reps=1):
    """Cached shard_map-jitted executor (mirrors bass2jax.run_bass_via_pjrt
    multi-core path) so repeated calls skip re-compilation."""
    if reps in _RUNNER:
        return _RUNNER[reps]
    import jax
    from jax.sharding import Mesh, PartitionSpec
    from jax.experimental.shard_map import shard_map
    from concourse import bass2jax

    nc = _build(reps)
    bass2jax.install_neuronx_cc_hook()

    part_name = nc.partition_id_tensor.name if nc.partition_id_tensor else None
    in_names, out_names, out_avals, zero_outs = [], [], [], []
    for alloc in nc.m.functions[0].allocations:
        if not isinstance(alloc, mybir.MemoryLocationSet):
            continue
        name = alloc.memorylocations[0].name
        if alloc.kind == "ExternalInput":
            if name != part_name:
                in_names.append(name)
        elif alloc.kind == "ExternalOutput":
            out_names.append(name)
            shape = tuple(alloc.tensor_shape)
            dtype = mybir.dt.np(alloc.dtype)
            out_avals.append(jax.core.ShapedArray(shape, dtype))
            zero_outs.append(np.zeros(shape, dtype))
    n_params = len(in_names)
    all_names = in_names + out_names
    if part_name is not None:
        all_names = all_names + [part_name]

    def _body(*args):
        operands = list(args)
        if part_name is not None:
            operands.append(bass2jax.partition_id_tensor())
        outs = bass2jax._bass_exec_p.bind(
            *operands,
            out_avals=tuple(out_avals),
            in_names=tuple(all_names),
            out_names=tuple(out_names),
            lowering_input_output_aliases=(),
            sim_require_finite=True,
            sim_require_nnan=True,
            nc=nc,
        )
        return tuple(outs)

    devices = jax.devices()[:NC]
    mesh = Mesh(np.asarray(devices), ("core",))
    n_outs = len(out_names)
    fn = jax.jit(
        shard_map(
            _body, mesh=mesh,
            in_specs=(PartitionSpec("core"),) * (n_params + n_outs),
            out_specs=(PartitionSpec("core"),) * n_outs,
            check_rep=False,
        ),
        donate_argnums=tuple(range(n_params, n_params + n_outs)),
        keep_unused=True,
    )

    def run(in_maps):
        concat_in = [
            np.concatenate([in_maps[c][nm] for c in range(NC)], axis=0)
            for nm in in_names
        ]
        concat_zeros = [
            np.zeros((NC * z.shape[0], *z.shape[1:]), z.dtype) for z in zero_outs
        ]
        out_arrs = fn(*concat_in, *concat_zeros)
        return [
            {nm: np.asarray(out_arrs[i]).reshape(NC, *out_avals[i].shape)[c]
             for i, nm in enumerate(out_names)}
            for c in range(NC)
        ]

    _RUNNER[reps] = run
    return run


def kernel(xyz, scales, rotations, velocities):
    run = _get_runner()
    in_maps = make_in_maps(xyz, scales, rotations, velocities)
    return finish(run(in_maps))


if __name__ == "__main__":
    rng = np.random.default_rng(0)
    ins = {
        "xyz": rng.standard_normal((B, N, 3)).astype(np.float32),
        "scales": rng.random((B, N, 3)).astype(np.float32),
        "rotations": rng.standard_normal((B, N, 4)).astype(np.float32),
        "velocities": rng.standard_normal((B, N, 3)).astype(np.float32),
    }
    print(kernel(**ins))
